# revision 1
# baseline (speedup 1.0000x reference)
"""Trainium2 Bass kernel for nn_Encoder_79843442033106 (retrieval_knn).

Reference computation:
  queries xq[b,k,:] (1024 x 2016, fp16 values) are matched against a codebook
  c (16001 x 2016) under squared L2 distance, searching the concatenation
  [d0, d1, d1, d0] where d0 = ||x-c||^2 and d1 = ||x-(1-c)||^2; the argmin
  index is emitted LSB-first as 32 bits -> output [64, 512] int32.

Identities used (per query q, code m; x2 = ||x||^2 is per-query and cannot
affect any argmin, so it is dropped everywhere):
  d0 - x2 =  c2[m] - 2*xc[q,m]          ( = -g0 )
  d1 - x2 = cn2[m] - 2*(xs[q] - xc[q,m])( = -g1 )
Blocks 2,3 of the reference concat are duplicates that can never win argmin
(first occurrence wins), so only d0/d1 are searched.

Device pipeline per core (codebook axis M sharded 8 ways, 2016 rows/core):
  * one fp16 GEMM psum[q,m] = sum_d 2*xq[d,q]*c[d,m] - c2[m], with the
    -c2 term folded into contraction K-tile 0 as two exact fp16 hi/lo rows
    (queries-side coefficient 1), so PSUM directly holds g0 = 2xc - c2.
  * DVE: v = (-psum) - s_rep      with s = c2 + cn2   (scalar_tensor_tensor)
         h = (v + 2xs) max psum   -> h[m] = max(g0, g1) = -(min(d0,d1)-x2)
         max8/max_index per 504-wide sub-chunk -> per-core candidates.
  * host merges the 32 candidates (max value, lowest-index tie-break),
    recovers which of d0/d1 won with one exact f64 dot per query, and
    emits the bits.
"""

import numpy as np

import concourse.bass as bass
import concourse.tile as tile
from concourse import bacc, mybir
from concourse.bass_utils import run_bass_kernel_spmd

# Problem constants (hardcoded per the harness contract).
B = 64
KSLOT = 16
D = 2016
M = 16001
NBITS = 32
BK = B * KSLOT           # 1024 queries
NCORES = 8
MLOC = 2016              # padded per-core codebook rows (8*2016 >= 16001)
KT = 126                 # contraction tile partitions (16 * 126 = 2016)
NK = D // KT             # 16 contraction tiles; tile 0 padded to 128 rows
DA = D + 2               # DRAM rows of the transposed operands (aug k=0)
NQT = BK // 128          # 8 query tiles
PAD_C2HI = np.float16(60000.0)   # g0 for padded codes ~ -60000: never wins
PAD_BIG = np.float32(1e30)       # s_rep padding: g1 ~ -1e30: never wins

_compiled = {}


def _ksl(k):
    """DRAM row slice of contraction tile k (tile 0 carries the 2 aug rows)."""
    return (0, 128) if k == 0 else (2 + k * KT, 2 + (k + 1) * KT)


def _build_program(repeat: int = 1) -> bass.Bass:
    """repeat>1 replays the whole compute body N times inside one NEFF —
    used only by bench.py to measure per-iteration device time
    differentially (dispatch overhead cancels)."""
    f16 = mybir.dt.float16
    f32 = mybir.dt.float32
    u32 = mybir.dt.uint32

    nc = bacc.Bacc("TRN2", debug=False, num_devices=NCORES)

    xqt = nc.dram_tensor("xqt", [DA, BK], f16, kind="ExternalInput").ap()
    ct = nc.dram_tensor("ct", [DA, MLOC], f16, kind="ExternalInput").ap()
    srep = nc.dram_tensor("srep", [128, MLOC], f32, kind="ExternalInput").ap()
    xs2 = nc.dram_tensor("xs2", [128, NQT], f32, kind="ExternalInput").ap()
    outv = nc.dram_tensor(
        "outv", [BK, 2, 2, 8], f32, kind="ExternalOutput"
    ).ap()
    outi = nc.dram_tensor(
        "outi", [BK, 2, 2, 8], u32, kind="ExternalOutput"
    ).ap()

    NP = 2           # m passes
    HM = MLOC // NP  # 1008 m-columns per pass

    with tile.TileContext(nc) as tc:
        with (
            tc.tile_pool(name="const", bufs=1) as const_pool,
            tc.tile_pool(name="psum", bufs=4, space="PSUM") as psum_pool,
            tc.tile_pool(name="work", bufs=4) as work_pool,
            tc.tile_pool(name="outs", bufs=3) as out_pool,
        ):
            # Two m-half passes: the first half (~4MB) streams in ahead
            # so all 8 q-tiles' GEMMs saturate the PE while the second
            # half (and srep) arrives behind them.
            xs2_t = const_pool.tile([128, NQT], f32, tag="xs2")
            srep_t = const_pool.tile([128, MLOC], f32, tag="srep")
            xq_tiles, ct_tiles = [], []
            for k in range(NK):
                s, e = _ksl(k)
                p = e - s
                tq = const_pool.tile([p, BK], f16, tag=f"xq{k}")
                tcb = const_pool.tile([p, MLOC], f16, tag=f"ct{k}")
                if k == 0:
                    # Small first pieces so q-tile 0's first matmul (which
                    # reads xq cols 0:128 and ct cols 0:512 only) starts
                    # after ~160KB of DMA instead of ~780KB.
                    nc.sync.dma_start(tq[:, 0:128], xqt[s:e, 0:128])
                    nc.sync.dma_start(tcb[:, 0:512], ct[s:e, 0:512])
                    nc.sync.dma_start(tq[:, 128:BK], xqt[s:e, 128:BK])
                    nc.sync.dma_start(tcb[:, 512:HM], ct[s:e, 512:HM])
                else:
                    nc.sync.dma_start(tq[:], xqt[s:e, :])
                    nc.sync.dma_start(tcb[:, 0:HM], ct[s:e, 0:HM])
                xq_tiles.append(tq)
                ct_tiles.append(tcb)
                if k == 0:
                    nc.sync.dma_start(xs2_t[:], xs2[:, :])
                if k == 2:
                    nc.sync.dma_start(srep_t[:, 0:HM], srep[:, 0:HM])
            for hp in range(1, NP):
                cs = hp * HM
                for k in range(NK):
                    s, e = _ksl(k)
                    nc.sync.dma_start(
                        ct_tiles[k][:, cs:cs + HM], ct[s:e, cs:cs + HM]
                    )
                nc.sync.dma_start(srep_t[:, cs:cs + HM], srep[:, cs:cs + HM])

            for rep in range(repeat):
              for hp in range(NP):
                cs = hp * HM
                for qt in range(NQT):
                    # GEMM: psum[q, m] = 2*xc - c2 (fp16 in, f32 accum).
                    # Chunks start on PSUM bank boundaries (512 f32): a
                    # matmul output may not straddle banks.
                    ps = psum_pool.tile([128, HM], f32, tag="ps")
                    for k in range(NK):
                        for lo, hi in ((0, 512), (512, HM)):
                            nc.tensor.matmul(
                                ps[:, lo:hi],
                                lhsT=xq_tiles[k][:, qt * 128:(qt + 1) * 128],
                                rhs=ct_tiles[k][:, cs + lo:cs + hi],
                                start=(k == 0),
                                stop=(k == NK - 1),
                            )

                    # ACT stages PSUM->SBUF (frees the PSUM slot fast and
                    # keeps the DVE ops off slow PSUM reads);
                    # DVE: v = (-t2) - s_rep ; h = (v + 2xs) max t2.
                    # All post-GEMM work runs on 504-wide sub-chunks so the
                    # dependency chain after the very last matmul (the
                    # kernel tail) stays short.
                    t2 = work_pool.tile([128, HM], f32, tag="t2")
                    v = work_pool.tile([128, HM], f32, tag="v")
                    h = work_pool.tile([128, HM], f32, tag="h")
                    for sub in range(2):
                        sl = slice(sub * 504, (sub + 1) * 504)
                        nc.scalar.copy(t2[:, sl], ps[:, sl])
                        nc.vector.scalar_tensor_tensor(
                            v[:, sl], in0=t2[:, sl], scalar=-1.0,
                            in1=srep_t[:, cs + sub * 504:cs + (sub + 1) * 504],
                            op0=mybir.AluOpType.mult,
                            op1=mybir.AluOpType.subtract,
                        )
                        nc.vector.scalar_tensor_tensor(
                            h[:, sl], in0=v[:, sl],
                            scalar=xs2_t[:, qt:qt + 1], in1=t2[:, sl],
                            op0=mybir.AluOpType.add, op1=mybir.AluOpType.max,
                        )
                        v_t = out_pool.tile([128, 8], f32, tag="v8")
                        nc.vector.max(v_t[:], h[:, sl])
                        i_t = out_pool.tile([128, 8], u32, tag="i8")
                        nc.vector.max_index(i_t[:], v_t[:], h[:, sl])
                        nc.sync.dma_start(
                            outv[qt * 128:(qt + 1) * 128, hp, sub, :], v_t[:]
                        )
                        nc.sync.dma_start(
                            outi[qt * 128:(qt + 1) * 128, hp, sub, :], i_t[:]
                        )

    nc.compile()
    return nc


def _host_prep(x: np.ndarray, data: np.ndarray):
    """Build per-core input maps: layout/shard prep plus the tiny norm
    vectors (c2/cn2 sums); all heavy FLOPs stay on device."""
    xq = np.transpose(
        x.reshape(B, 2, 126, KSLOT, 8), (0, 3, 1, 2, 4)
    ).reshape(BK, D)
    # [DA, BK]: rows 0,1 are the aug coefficient rows (ones); rows 2..
    # are 2*xq transposed (exact fp16 scaling).
    xqt2 = np.empty((DA, BK), dtype=np.float16)
    xqt2[0:2] = 1.0
    xqt2[2:] = (xq.astype(np.float16) * np.float16(2.0)).T

    xq64 = xq.astype(np.float64)
    xs2 = np.ascontiguousarray(
        (2.0 * xq64.sum(axis=1)).astype(np.float32).reshape(NQT, 128).T
    )

    c = data.reshape(M, D)
    c64 = c.astype(np.float64)
    c2_all = np.einsum("md,md->m", c64, c64)
    # cn2 = sum((1-c)^2) = D - 2*sum(c) + c2, exact in f64.
    cn2_all = D - 2.0 * c64.sum(axis=1) + c2_all

    in_maps = []
    for core in range(NCORES):
        s = core * MLOC
        e = min(s + MLOC, M)
        n = e - s
        ct = np.zeros((DA, MLOC), dtype=np.float16)
        ct[2:, :n] = c[s:e].T
        # Exact fp16 hi/lo split of -c2 in the two aug rows.
        c2_hi = np.full(MLOC, -PAD_C2HI, dtype=np.float16)
        c2_hi[:n] = -c2_all[s:e].astype(np.float16)
        c2_lo = np.zeros(MLOC, dtype=np.float16)
        c2_lo[:n] = -(c2_all[s:e] + c2_hi[:n].astype(np.float64))
        ct[0] = c2_hi
        ct[1] = c2_lo
        srep = np.full(MLOC, PAD_BIG, dtype=np.float32)
        srep[:n] = (c2_all[s:e] + cn2_all[s:e]).astype(np.float32)
        in_maps.append({
            "xqt": xqt2,
            "ct": np.ascontiguousarray(ct),
            "srep": np.ascontiguousarray(
                np.broadcast_to(srep[None, :], (128, MLOC))
            ),
            "xs2": xs2,
        })
    return in_maps


def _merge(results, x: np.ndarray, data: np.ndarray):
    """Merge per-core top-1 candidates; recover the d0/d1 side with one
    exact f64 dot per query."""
    # Candidates: per core, top-1 of each 504-wide m sub-chunk.
    vals = np.stack([r["outv"][:, :, :, 0] for r in results])  # [8,1024,2,2]
    ms = np.stack(
        [r["outi"][:, :, :, 0].astype(np.int64) for r in results]
    )
    ms = ms + np.arange(4, dtype=np.int64).reshape(1, 1, 2, 2) * 504
    vals = np.transpose(vals, (0, 2, 3, 1)).reshape(4 * NCORES, BK)
    ms = np.transpose(ms, (0, 2, 3, 1)).reshape(4 * NCORES, BK)

    best = np.argmax(vals, axis=0)                                 # [1024]
    q = np.arange(BK)
    r_win = (best // 4) * MLOC + ms[best, q]                       # code row

    xq = np.transpose(
        x.reshape(B, 2, 126, KSLOT, 8), (0, 3, 1, 2, 4)
    ).reshape(BK, D).astype(np.float64)
    cwin = data.reshape(M, D)[r_win].astype(np.float64)              # [1024,D]
    dot = np.einsum("qd,qd->q", xq, cwin)
    xs = xq.sum(axis=1)
    # d0 - d1 = c2 - cn2 - 2*(2*dot - xs); side 0 wins ties.
    c2 = (cwin * cwin).sum(axis=1)
    cn2 = ((1.0 - cwin) ** 2).sum(axis=1)
    side = (c2 - 2.0 * dot > cn2 - 2.0 * (xs - dot)).astype(np.int64)
    return r_win + side * M                                          # [1024]


def kernel(x: np.ndarray, data: np.ndarray) -> np.ndarray:
    if "nc" not in _compiled:
        _compiled["nc"] = _build_program()
    nc = _compiled["nc"]

    x = np.asarray(x)
    data = np.asarray(data)
    in_maps = _host_prep(x, data)
    res = run_bass_kernel_spmd(nc, in_maps, list(range(NCORES)))
    _compiled["last_result"] = res

    g = _merge(res.results, x, data).astype(np.int32)                # [1024]
    shifts = np.arange(NBITS, dtype=np.int32)
    bits = (g[:, None] >> shifts[None, :]) & 1
    return bits.astype(np.int32).reshape(B, KSLOT * NBITS)

